# revision 10
# baseline (speedup 1.0000x reference)
"""Trainium2 Bass kernel for nn_BroadcastRouter (GNN message passing).

Computation (per region r of R=4096, B=16, D=256, N=16 neighbors, top-K=4):
  sims[r, n]  = mean over B*D of feats[r] * feats[nbr[r, n]]
  sel         = indices of top-4 sims (stable, jax.lax.top_k tie-breaking)
  agg[r]      = mean_k bcast[nbr[r, sel_k]]
  out[r]      = concat([bcast[r], agg[r]]) @ mix_w.T + mix_b

Distribution: regions sharded across 8 cores (512 each); the full feats/bcast
tables are replicated into every core's HBM so neighbor gathers are local
indirect DMAs (no collectives needed).

Per-core pipeline, blocks of 128 regions (phases software-pipelined so the
bcast gathers of block b preempt the feats gathers of block b+1, and the
matmul tail of block b runs under the sims phase of later blocks):
  A: load local rows; 16x indirect row-gather of neighbor feats; fused
     multiply+accumulate on DVE -> sims [128, 16]; PE-transpose local bcast
  B: exact stable rank of each candidate (counting comparisons) -> top-4
  C: gather the 4 selected bcast rows, sum on DVE (0.25 folded into weights)
  D: PE-transpose agg, tiny mix matmul vs pre-transposed weights (bias via a
     K=1 matmul), DMA out.
"""

import numpy as np

R, B, D, N, TOP_K = 4096, 16, 256, 16, 4
NCORES = 8
BD = B * D
P = 128
EC = D // P  # 2 e-chunks of 128 per half


def build(r_total=R, n_cores=NCORES, debug=False):
    import concourse.bass as bass
    import concourse.bacc as bacc
    import concourse.mybir as mybir
    import concourse.tile as tile
    from concourse.masks import make_identity

    f32 = mybir.dt.float32
    i32 = mybir.dt.int32
    Alu = mybir.AluOpType

    rl = r_total // n_cores
    nblk = rl // P
    assert rl % P == 0

    nc = bacc.Bacc("TRN2", target_bir_lowering=False, debug=False,
                   num_devices=n_cores)
    feats = nc.dram_tensor("feats", [r_total, BD], f32, kind="ExternalInput")
    bcast = nc.dram_tensor("bcast", [r_total, BD], f32, kind="ExternalInput")
    featsL = nc.dram_tensor("feats_local", [rl, BD], f32, kind="ExternalInput")
    bcastL = nc.dram_tensor("bcast_local", [rl, BD], f32, kind="ExternalInput")
    nbrL = nc.dram_tensor("nbr_local", [rl, N], i32, kind="ExternalInput")
    # w1t/w2t are [e, d] = mix_w[:, :D].T and 0.25 * mix_w[:, D:].T
    w1t = nc.dram_tensor("w1t", [D, D], f32, kind="ExternalInput")
    w2t = nc.dram_tensor("w2t", [D, D], f32, kind="ExternalInput")
    biasw = nc.dram_tensor("biasw", [1, D], f32, kind="ExternalInput")
    outL = nc.dram_tensor("out_local", [rl * B, D], f32, kind="ExternalOutput")
    if debug:
        dbg_sims = nc.dram_tensor("dbg_sims", [rl, N], f32, kind="ExternalOutput")
        dbg_rank = nc.dram_tensor("dbg_rank", [rl, N], f32, kind="ExternalOutput")
        dbg_sel = nc.dram_tensor("dbg_sel", [rl, TOP_K], i32, kind="ExternalOutput")
        dbg_ag = nc.dram_tensor("dbg_ag", [rl, BD], f32, kind="ExternalOutput")

    with tile.TileContext(nc) as tc:
        with (
            tc.tile_pool(name="const", bufs=1) as const,
            tc.tile_pool(name="big", bufs=11) as big,
            tc.tile_pool(name="small", bufs=3) as small,
            tc.tile_pool(name="outp", bufs=3) as outp,
            tc.tile_pool(name="psum", bufs=4, space="PSUM") as psum,
            tc.tile_pool(name="psmm", bufs=2, space="PSUM") as psmm,
        ):
            ident = const.tile([P, P], f32, tag="ident")
            make_identity(nc, ident[:])
            ones1 = const.tile([1, P], f32, tag="ones")
            nc.gpsimd.memset(ones1[:], 1.0)
            w1sb = const.tile([P, EC, D], f32, tag="w1")
            w2sb = const.tile([P, EC, D], f32, tag="w2")
            for ec in range(EC):
                nc.sync.dma_start(out=w1sb[:, ec, :], in_=w1t[ec * P:(ec + 1) * P, :])
                nc.sync.dma_start(out=w2sb[:, ec, :], in_=w2t[ec * P:(ec + 1) * P, :])
            bsb = const.tile([1, D], f32, tag="bias")
            nc.sync.dma_start(out=bsb[:], in_=biasw[:])

            st = [dict() for _ in range(nblk)]

            def transpose_to(src, dstT):
                # layout: dstT[p=e_low, (ec, r, b)] so that a 128-token tile
                # (8 regions x 16 b, m = r*B + b) is a contiguous slice.
                dv = dstT[:].rearrange("p (ec r b) -> p ec r b", ec=EC, b=B)
                for b_i in range(B):
                    for ec in range(EC):
                        pt = psum.tile([P, P], f32, tag="tr")
                        off = b_i * D + ec * P
                        nc.tensor.transpose(out=pt[:], in_=src[:, off:off + P],
                                            identity=ident[:])
                        nc.scalar.copy(out=dv[:, ec, :, b_i], in_=pt[:])

            def phase_a(blk):
                """loads + neighbor-feature gathers + fused sims; local-bcast
                transposes run here too (PE/ACT are idle in the gather phase)."""
                s = st[blk]
                r0 = blk * P
                idx_t = small.tile([P, N], i32, tag="idx")
                nc.sync.dma_start(out=idx_t[:], in_=nbrL[r0:r0 + P, :])
                L_t = big.tile([P, BD], f32, tag="big")
                nc.sync.dma_start(out=L_t[:], in_=featsL[r0:r0 + P, :])
                BL = big.tile([P, BD], f32, tag="big")
                nc.sync.dma_start(out=BL[:], in_=bcastL[r0:r0 + P, :])
                sims = small.tile([P, N], f32, tag="sims")
                junk = big.tile([P, BD], f32, tag="big")
                for n in range(N):
                    G = big.tile([P, BD], f32, tag="big")
                    nc.gpsimd.indirect_dma_start(
                        out=G[:], out_offset=None, in_=feats[:],
                        in_offset=bass.IndirectOffsetOnAxis(
                            ap=idx_t[:, n:n + 1], axis=0),
                    )
                    # junk = G * L ; sims[:, n] = sum(junk)
                    nc.vector.scalar_tensor_tensor(
                        out=junk[:], in0=G[:], scalar=0.0, in1=L_t[:],
                        op0=Alu.bypass, op1=Alu.mult,
                        accum_out=sims[:, n:n + 1],
                    )
                BLT = big.tile([P, BD], f32, tag="big")
                transpose_to(BL, BLT)
                s.update(idx_t=idx_t, sims=sims, BLT=BLT, r0=r0)

            def phase_b(blk):
                """exact stable rank (jax.lax.top_k tie-break) -> selected idx."""
                s = st[blk]
                sims, idx_t = s["sims"], s["idx_t"]
                nbrf = small.tile([P, N], f32, tag="nbrf")
                nc.vector.tensor_copy(out=nbrf[:], in_=idx_t[:])
                cnt = small.tile([P, N], f32, tag="cnt")
                cnteq = small.tile([P, N], f32, tag="cnteq")
                junk16 = small.tile([P, N], f32, tag="junk16")
                nc.vector.memset(cnteq[:, 0:1], 0.0)
                for p in range(N):
                    # cnt[:, p] = #{q : sims[q] > sims[p]}
                    nc.vector.tensor_scalar(
                        out=junk16[:], in0=sims[:], scalar1=sims[:, p:p + 1],
                        scalar2=None, op0=Alu.is_gt, op1=Alu.add,
                        accum_out=cnt[:, p:p + 1],
                    )
                    if p > 0:
                        # cnteq[:, p] = #{q < p : sims[q] == sims[p]}
                        nc.vector.tensor_scalar(
                            out=junk16[:, 0:p], in0=sims[:, 0:p],
                            scalar1=sims[:, p:p + 1], scalar2=None,
                            op0=Alu.is_equal, op1=Alu.add,
                            accum_out=cnteq[:, p:p + 1],
                        )
                rank = small.tile([P, N], f32, tag="rank")
                nc.vector.tensor_tensor(out=rank[:], in0=cnt[:], in1=cnteq[:],
                                        op=Alu.add)
                # sel_k = neighbor index whose rank == k (unique by construction)
                self_f = small.tile([P, TOP_K], f32, tag="self")
                for k in range(TOP_K):
                    nc.vector.scalar_tensor_tensor(
                        out=junk16[:], in0=rank[:], scalar=float(k), in1=nbrf[:],
                        op0=Alu.is_equal, op1=Alu.mult,
                        accum_out=self_f[:, k:k + 1],
                    )
                sel_i = small.tile([P, TOP_K], i32, tag="seli")
                nc.vector.tensor_copy(out=sel_i[:], in_=self_f[:])
                s.update(sel_i=sel_i, rank=rank)

            def phase_c(blk):
                """gather the 4 selected bcast rows, sum (0.25 folded in w2t)."""
                s = st[blk]
                sel_i = s["sel_i"]
                AG = big.tile([P, BD], f32, tag="big")
                Bk = big.tile([P, BD], f32, tag="big")
                for k in range(TOP_K):
                    dst = AG if k == 0 else Bk
                    nc.gpsimd.indirect_dma_start(
                        out=dst[:], out_offset=None, in_=bcast[:],
                        in_offset=bass.IndirectOffsetOnAxis(
                            ap=sel_i[:, k:k + 1], axis=0),
                    )
                    if k > 0:
                        nc.vector.tensor_tensor(out=AG[:], in0=AG[:], in1=Bk[:],
                                                op=Alu.add)
                s.update(AG=AG)

            def phase_d(blk):
                """transpose agg, final mix matmuls + bias, write out."""
                s = st[blk]
                r0, AG, BLT = s["r0"], s["AG"], s["BLT"]
                if debug:
                    nc.sync.dma_start(out=dbg_sims[r0:r0 + P, :], in_=s["sims"][:])
                    nc.sync.dma_start(out=dbg_rank[r0:r0 + P, :], in_=s["rank"][:])
                    nc.sync.dma_start(out=dbg_sel[r0:r0 + P, :], in_=s["sel_i"][:])
                    nc.sync.dma_start(out=dbg_ag[r0:r0 + P, :], in_=AG[:])
                AGT = big.tile([P, BD], f32, tag="big")
                transpose_to(AG, AGT)
                ntt = P // B  # 8 regions per token tile
                for tt in range(P // ntt):
                    ps = psmm.tile([P, D], f32, tag="mm")
                    first = True
                    for srcT, wsb in ((BLT, w1sb), (AGT, w2sb)):
                        for ec in range(EC):
                            off = ec * (P * B) + tt * P
                            lhsT = srcT[:, off:off + P]
                            nc.tensor.matmul(out=ps[:], lhsT=lhsT,
                                             rhs=wsb[:, ec, :],
                                             start=first, stop=False)
                            first = False
                    nc.tensor.matmul(out=ps[:], lhsT=ones1[:1, :], rhs=bsb[:1, :],
                                     start=False, stop=True)
                    ot = outp.tile([P, D], f32, tag="ot")
                    nc.scalar.copy(out=ot[:], in_=ps[:])
                    row0 = (r0 + tt * ntt) * B
                    # out-stores ride the ACT HWDGE ring so they never sit in
                    # front of the next block's loads on the sync ring.
                    nc.scalar.dma_start(out=outL[row0:row0 + P, :], in_=ot[:])

            # software-pipelined emission: C(b) preempts A(b+1)'s gathers on
            # the gpsimd queue; D(b) overlaps the next sims phase.
            ph = {"a": phase_a, "b": phase_b, "c": phase_c, "d": phase_d}
            sched = []
            for b in range(nblk):
                sched += [("a", b), ("b", b), ("c", b), ("d", b)]
            for name, b in sched:
                ph[name](b)

    nc.compile()
    return nc


_CACHE = {}


def _get_nc():
    if "nc" not in _CACHE:
        _CACHE["nc"] = build()
    return _CACHE["nc"]


def _prep_in_maps(bcast_by_region, feats_by_region, neighbor_indices, mix_w,
                  mix_b):
    f2 = np.ascontiguousarray(
        np.asarray(feats_by_region, dtype=np.float32).reshape(R, BD))
    bc = np.ascontiguousarray(
        np.asarray(bcast_by_region, dtype=np.float32).reshape(R, BD))
    nbr = np.ascontiguousarray(np.asarray(neighbor_indices, dtype=np.int32))
    mw = np.asarray(mix_w, dtype=np.float32)
    mb = np.asarray(mix_b, dtype=np.float32)
    w1t = np.ascontiguousarray(mw[:, :D].T)
    w2t = np.ascontiguousarray(mw[:, D:].T) * np.float32(1.0 / TOP_K)
    biasw = np.ascontiguousarray(mb.reshape(1, D))

    rl = R // NCORES
    in_maps = []
    for c in range(NCORES):
        in_maps.append({
            "feats": f2,
            "bcast": bc,
            "feats_local": np.ascontiguousarray(f2[c * rl:(c + 1) * rl]),
            "bcast_local": np.ascontiguousarray(bc[c * rl:(c + 1) * rl]),
            "nbr_local": np.ascontiguousarray(nbr[c * rl:(c + 1) * rl]),
            "w1t": w1t,
            "w2t": w2t,
            "biasw": biasw,
        })
    return in_maps


def run(in_maps, **kwargs):
    from concourse.bass_utils import run_bass_kernel_spmd

    nc = _get_nc()
    return run_bass_kernel_spmd(nc, in_maps, list(range(NCORES)), **kwargs)


def assemble(res):
    rl = R // NCORES
    return np.concatenate(
        [res.results[c]["out_local"].reshape(rl, B, D) for c in range(NCORES)],
        axis=0)


def kernel(bcast_by_region, feats_by_region, neighbor_indices, mix_w, mix_b):
    in_maps = _prep_in_maps(bcast_by_region, feats_by_region,
                            neighbor_indices, mix_w, mix_b)
    return assemble(run(in_maps))


# revision 15
# speedup vs baseline: 1.0760x; 1.0760x over previous
"""Trainium2 Bass kernel for nn_BroadcastRouter (GNN message passing).

Computation (per region r of R=4096, B=16, D=256, N=16 neighbors, top-K=4):
  sims[r, n]  = mean over B*D of feats[r] * feats[nbr[r, n]]
  sel         = indices of top-4 sims (stable, jax.lax.top_k tie-breaking)
  agg[r]      = mean_k bcast[nbr[r, sel_k]]
  out[r]      = concat([bcast[r], agg[r]]) @ mix_w.T + mix_b

Distribution: regions sharded across 8 cores (512 each); the full feats/bcast
tables are replicated into every core's HBM so neighbor gathers are local
indirect DMAs (no collectives needed).

Per-core pipeline, blocks of 128 regions (phases software-pipelined so the
bcast gathers of block b preempt the feats gathers of block b+1, and the
matmul tail of block b runs under the sims phase of later blocks):
  A: load local rows; 16x indirect row-gather of neighbor feats; fused
     multiply+accumulate on DVE -> sims [128, 16]; PE-transpose local bcast
  B: exact stable rank of each candidate (counting comparisons) -> top-4
  C: gather the 4 selected bcast rows, sum on DVE (0.25 folded into weights)
  D: PE-transpose agg, tiny mix matmul vs pre-transposed weights (bias via a
     K=1 matmul), DMA out.
"""

import numpy as np

R, B, D, N, TOP_K = 4096, 16, 256, 16, 4
NCORES = 8
BD = B * D
P = 128
EC = D // P  # 2 e-chunks of 128 per half


def build(r_total=R, n_cores=NCORES, debug=False):
    import concourse.bass as bass
    import concourse.bacc as bacc
    import concourse.mybir as mybir
    import concourse.tile as tile
    from concourse.masks import make_identity

    f32 = mybir.dt.float32
    i32 = mybir.dt.int32
    Alu = mybir.AluOpType

    rl = r_total // n_cores
    nblk = rl // P
    assert rl % P == 0

    nc = bacc.Bacc("TRN2", target_bir_lowering=False, debug=False,
                   num_devices=n_cores)
    feats = nc.dram_tensor("feats", [r_total, BD], f32, kind="ExternalInput")
    bcast = nc.dram_tensor("bcast", [r_total, BD], f32, kind="ExternalInput")
    featsL = nc.dram_tensor("feats_local", [rl, BD], f32, kind="ExternalInput")
    bcastL = nc.dram_tensor("bcast_local", [rl, BD], f32, kind="ExternalInput")
    nbrL = nc.dram_tensor("nbr_local", [rl, N], i32, kind="ExternalInput")
    # w1t/w2t are [e, d] = mix_w[:, :D].T and 0.25 * mix_w[:, D:].T
    w1t = nc.dram_tensor("w1t", [D, D], f32, kind="ExternalInput")
    w2t = nc.dram_tensor("w2t", [D, D], f32, kind="ExternalInput")
    biasw = nc.dram_tensor("biasw", [1, D], f32, kind="ExternalInput")
    outL = nc.dram_tensor("out_local", [rl * B, D], f32, kind="ExternalOutput")
    if debug:
        dbg_sims = nc.dram_tensor("dbg_sims", [rl, N], f32, kind="ExternalOutput")
        dbg_rank = nc.dram_tensor("dbg_rank", [rl, N], f32, kind="ExternalOutput")
        dbg_sel = nc.dram_tensor("dbg_sel", [rl, TOP_K], i32, kind="ExternalOutput")
        dbg_ag = nc.dram_tensor("dbg_ag", [rl, BD], f32, kind="ExternalOutput")

    with tile.TileContext(nc) as tc:
        with (
            tc.tile_pool(name="const", bufs=1) as const,
            tc.tile_pool(name="big", bufs=11) as big,
            tc.tile_pool(name="small", bufs=3) as small,
            tc.tile_pool(name="outp", bufs=3) as outp,
            tc.tile_pool(name="psum", bufs=4, space="PSUM") as psum,
            tc.tile_pool(name="psmm", bufs=3, space="PSUM") as psmm,
        ):
            ident = const.tile([P, P], f32, tag="ident")
            make_identity(nc, ident[:])
            ones1 = const.tile([1, P], f32, tag="ones")
            nc.gpsimd.memset(ones1[:], 1.0)
            w1sb = const.tile([P, EC, D], f32, tag="w1")
            w2sb = const.tile([P, EC, D], f32, tag="w2")
            for ec in range(EC):
                nc.sync.dma_start(out=w1sb[:, ec, :], in_=w1t[ec * P:(ec + 1) * P, :])
                nc.sync.dma_start(out=w2sb[:, ec, :], in_=w2t[ec * P:(ec + 1) * P, :])
            bsb = const.tile([1, D], f32, tag="bias")
            nc.sync.dma_start(out=bsb[:], in_=biasw[:])

            st = [dict() for _ in range(nblk)]

            def transpose_to(src, dstT, copy_eng=None):
                # layout: dstT[p=e_low, (ec, r, b)] so that a 128-token tile
                # (8 regions x 16 b, m = r*B + b) is a contiguous slice.
                dv = dstT[:].rearrange("p (ec r b) -> p ec r b", ec=EC, b=B)
                for b_i in range(B):
                    for ec in range(EC):
                        pt = psum.tile([P, P], f32, tag="tr")
                        off = b_i * D + ec * P
                        nc.tensor.transpose(out=pt[:], in_=src[:, off:off + P],
                                            identity=ident[:])
                        if copy_eng is None:
                            nc.scalar.copy(out=dv[:, ec, :, b_i], in_=pt[:])
                        else:
                            copy_eng.tensor_copy(out=dv[:, ec, :, b_i], in_=pt[:])

            def phase_a(blk):
                """loads + neighbor-feature gathers + fused sims; local-bcast
                transposes run here too (PE/ACT are idle in the gather phase)."""
                s = st[blk]
                r0 = blk * P
                idx_t = small.tile([P, N], i32, tag="idx")
                nc.sync.dma_start(out=idx_t[:], in_=nbrL[r0:r0 + P, :])
                L_t = big.tile([P, BD], f32, tag="big")
                nc.sync.dma_start(out=L_t[:], in_=featsL[r0:r0 + P, :])
                BL = big.tile([P, BD], f32, tag="big")
                nc.sync.dma_start(out=BL[:], in_=bcastL[r0:r0 + P, :])
                sims = small.tile([P, N], f32, tag="sims")
                junk = big.tile([P, BD], f32, tag="big")
                for n in range(N):
                    G = big.tile([P, BD], f32, tag="big")
                    nc.gpsimd.indirect_dma_start(
                        out=G[:], out_offset=None, in_=feats[:],
                        in_offset=bass.IndirectOffsetOnAxis(
                            ap=idx_t[:, n:n + 1], axis=0),
                    )
                    # junk = G * L ; sims[:, n] = sum(junk)
                    nc.vector.scalar_tensor_tensor(
                        out=junk[:], in0=G[:], scalar=0.0, in1=L_t[:],
                        op0=Alu.bypass, op1=Alu.mult,
                        accum_out=sims[:, n:n + 1],
                    )
                BLT = big.tile([P, BD], f32, tag="big")
                transpose_to(BL, BLT)
                s.update(idx_t=idx_t, sims=sims, BLT=BLT, r0=r0)

            def phase_b(blk):
                """exact stable rank (jax.lax.top_k tie-break) -> selected idx."""
                s = st[blk]
                sims, idx_t = s["sims"], s["idx_t"]
                nbrf = small.tile([P, N], f32, tag="nbrf")
                nc.vector.tensor_copy(out=nbrf[:], in_=idx_t[:])
                cnt = small.tile([P, N], f32, tag="cnt")
                cnteq = small.tile([P, N], f32, tag="cnteq")
                junk16 = small.tile([P, N], f32, tag="junk16")
                nc.vector.memset(cnteq[:, 0:1], 0.0)
                for p in range(N):
                    # cnt[:, p] = #{q : sims[q] > sims[p]}
                    nc.vector.tensor_scalar(
                        out=junk16[:], in0=sims[:], scalar1=sims[:, p:p + 1],
                        scalar2=None, op0=Alu.is_gt, op1=Alu.add,
                        accum_out=cnt[:, p:p + 1],
                    )
                    if p > 0:
                        # cnteq[:, p] = #{q < p : sims[q] == sims[p]}
                        nc.vector.tensor_scalar(
                            out=junk16[:, 0:p], in0=sims[:, 0:p],
                            scalar1=sims[:, p:p + 1], scalar2=None,
                            op0=Alu.is_equal, op1=Alu.add,
                            accum_out=cnteq[:, p:p + 1],
                        )
                rank = small.tile([P, N], f32, tag="rank")
                nc.vector.tensor_tensor(out=rank[:], in0=cnt[:], in1=cnteq[:],
                                        op=Alu.add)
                # sel_k = neighbor index whose rank == k (unique by construction)
                self_f = small.tile([P, TOP_K], f32, tag="self")
                for k in range(TOP_K):
                    nc.vector.scalar_tensor_tensor(
                        out=junk16[:], in0=rank[:], scalar=float(k), in1=nbrf[:],
                        op0=Alu.is_equal, op1=Alu.mult,
                        accum_out=self_f[:, k:k + 1],
                    )
                sel_i = small.tile([P, TOP_K], i32, tag="seli")
                nc.vector.tensor_copy(out=sel_i[:], in_=self_f[:])
                s.update(sel_i=sel_i, rank=rank)

            def phase_c(blk):
                """gather the 4 selected bcast rows, sum (0.25 folded in w2t)."""
                s = st[blk]
                sel_i = s["sel_i"]
                AG = big.tile([P, BD], f32, tag="big")
                Bk = big.tile([P, BD], f32, tag="big")
                for k in range(TOP_K):
                    dst = AG if k == 0 else Bk
                    nc.gpsimd.indirect_dma_start(
                        out=dst[:], out_offset=None, in_=bcast[:],
                        in_offset=bass.IndirectOffsetOnAxis(
                            ap=sel_i[:, k:k + 1], axis=0),
                    )
                    if k > 0:
                        nc.vector.tensor_tensor(out=AG[:], in0=AG[:], in1=Bk[:],
                                                op=Alu.add)
                s.update(AG=AG)

            def phase_d(blk):
                """transpose agg, final mix matmuls + bias, write out."""
                s = st[blk]
                r0, AG, BLT = s["r0"], s["AG"], s["BLT"]
                if debug:
                    nc.sync.dma_start(out=dbg_sims[r0:r0 + P, :], in_=s["sims"][:])
                    nc.sync.dma_start(out=dbg_rank[r0:r0 + P, :], in_=s["rank"][:])
                    nc.sync.dma_start(out=dbg_sel[r0:r0 + P, :], in_=s["sel_i"][:])
                    nc.sync.dma_start(out=dbg_ag[r0:r0 + P, :], in_=AG[:])
                AGT = big.tile([P, BD], f32, tag="big")
                transpose_to(AG, AGT, copy_eng=nc.vector)
                ntt = P // B  # 8 regions per token tile
                for tt in range(P // ntt):
                    ps = psmm.tile([P, D], f32, tag="mm")
                    first = True
                    for srcT, wsb in ((BLT, w1sb), (AGT, w2sb)):
                        for ec in range(EC):
                            off = ec * (P * B) + tt * P
                            lhsT = srcT[:, off:off + P]
                            nc.tensor.matmul(out=ps[:], lhsT=lhsT,
                                             rhs=wsb[:, ec, :],
                                             start=first, stop=False)
                            first = False
                    nc.tensor.matmul(out=ps[:], lhsT=ones1[:1, :], rhs=bsb[:1, :],
                                     start=False, stop=True)
                    ot = outp.tile([P, D], f32, tag="ot")
                    nc.scalar.copy(out=ot[:], in_=ps[:])
                    row0 = (r0 + tt * ntt) * B
                    # out-stores ride the ACT HWDGE ring so they never sit in
                    # front of the next block's loads on the sync ring.
                    nc.scalar.dma_start(out=outL[row0:row0 + P, :], in_=ot[:])

            # software-pipelined emission: C(b) preempts A(b+1)'s gathers on
            # the gpsimd queue; D(b) overlaps the next sims phase.
            ph = {"a": phase_a, "b": phase_b, "c": phase_c, "d": phase_d}
            sched = []
            for b in range(nblk):
                sched += [("a", b), ("b", b), ("c", b)]
                if b >= 1:
                    sched += [("d", b - 1)]
            sched += [("d", nblk - 1)]
            for name, b in sched:
                ph[name](b)

    nc.compile()
    return nc


_CACHE = {}


def _get_nc():
    if "nc" not in _CACHE:
        _CACHE["nc"] = build()
    return _CACHE["nc"]


def _prep_in_maps(bcast_by_region, feats_by_region, neighbor_indices, mix_w,
                  mix_b):
    f2 = np.ascontiguousarray(
        np.asarray(feats_by_region, dtype=np.float32).reshape(R, BD))
    bc = np.ascontiguousarray(
        np.asarray(bcast_by_region, dtype=np.float32).reshape(R, BD))
    nbr = np.ascontiguousarray(np.asarray(neighbor_indices, dtype=np.int32))
    mw = np.asarray(mix_w, dtype=np.float32)
    mb = np.asarray(mix_b, dtype=np.float32)
    w1t = np.ascontiguousarray(mw[:, :D].T)
    w2t = np.ascontiguousarray(mw[:, D:].T) * np.float32(1.0 / TOP_K)
    biasw = np.ascontiguousarray(mb.reshape(1, D))

    rl = R // NCORES
    in_maps = []
    for c in range(NCORES):
        in_maps.append({
            "feats": f2,
            "bcast": bc,
            "feats_local": np.ascontiguousarray(f2[c * rl:(c + 1) * rl]),
            "bcast_local": np.ascontiguousarray(bc[c * rl:(c + 1) * rl]),
            "nbr_local": np.ascontiguousarray(nbr[c * rl:(c + 1) * rl]),
            "w1t": w1t,
            "w2t": w2t,
            "biasw": biasw,
        })
    return in_maps


def run(in_maps, **kwargs):
    from concourse.bass_utils import run_bass_kernel_spmd

    nc = _get_nc()
    return run_bass_kernel_spmd(nc, in_maps, list(range(NCORES)), **kwargs)


def assemble(res):
    rl = R // NCORES
    return np.concatenate(
        [res.results[c]["out_local"].reshape(rl, B, D) for c in range(NCORES)],
        axis=0)


def kernel(bcast_by_region, feats_by_region, neighbor_indices, mix_w, mix_b):
    in_maps = _prep_in_maps(bcast_by_region, feats_by_region,
                            neighbor_indices, mix_w, mix_b)
    return assemble(run(in_maps))
